# revision 1
# baseline (speedup 1.0000x reference)
"""Bahdanau attention Trainium2 kernel.

Reference computation (per batch b):
    S_    = S[b] @ W_w.T + W_b          # [LS, D2]
    score = S_ @ H[b].T                 # [LS, LH]
    P     = softmax(score + pad_mask[b], axis=-1)
    out   = P @ H[b]                    # [LS, D2]

Sharding: data-parallel over batch B=16 across 8 NeuronCores (2 batches/core),
W replicated. All matmuls run in fp16 (full PE rate) with fp32 PSUM
accumulation; softmax statistics in fp32.

Host-side prep (inside kernel()): shard, cast to fp16, pre-transpose
S -> S^T (contraction over d needs d on partitions), W -> W^T, H -> H^T
(mm2 rhs), and keep H natural (mm3 rhs). The softmax probabilities are
transposed on-chip via PE transposes.
"""

import numpy as np

B, L, D = 16, 1024, 1024
NCORES = 8
BPC = B // NCORES  # batches per core
P = 128
NCH = D // P  # 128-row chunks per 1024 dim
SC = 512  # s-chunk width for the projection matmul
NTILE = BPC * (L // P)  # s-tiles per core across batches

_nc_cache = {}


def _build_nc(with_mask: bool, with_bias: bool):
    from contextlib import ExitStack

    import concourse.tile as tile
    from concourse import bacc, mybir
    from concourse.masks import make_identity

    f16 = mybir.dt.float16
    f32 = mybir.dt.float32
    AX = mybir.AxisListType.X
    EXP = mybir.ActivationFunctionType.Exp

    nc = bacc.Bacc("TRN2", target_bir_lowering=False, debug=False,
                   num_devices=NCORES)

    sT = nc.dram_tensor("sT", [BPC, D, L], f16, kind="ExternalInput").ap()
    hT = nc.dram_tensor("hT", [BPC, D, L], f16, kind="ExternalInput").ap()
    h_ = nc.dram_tensor("h", [BPC, L, D], f16, kind="ExternalInput").ap()
    wT = nc.dram_tensor("wT", [D, D], f16, kind="ExternalInput").ap()
    wb = (nc.dram_tensor("wb", [P, NCH], f32, kind="ExternalInput").ap()
          if with_bias else None)
    msk = (nc.dram_tensor("mask", [BPC, L, L], f32, kind="ExternalInput").ap()
           if with_mask else None)
    out = nc.dram_tensor("out", [BPC, L, D], f32, kind="ExternalOutput").ap()

    with tile.TileContext(nc) as tc, ExitStack() as ctx:
        ep = ctx.enter_context
        singles = ep(tc.tile_pool(name="singles", bufs=1))
        batchp = ep(tc.tile_pool(name="batchp", bufs=2))
        sinp = ep(tc.tile_pool(name="sin", bufs=2))
        projp = ep(tc.tile_pool(name="proj", bufs=2))
        pbuf = ep(tc.tile_pool(name="pbuf", bufs=2))
        ptbuf = ep(tc.tile_pool(name="ptbuf", bufs=2))
        outp = ep(tc.tile_pool(name="outp", bufs=3))
        statp = ep(tc.tile_pool(name="statp", bufs=4))
        maskp = ep(tc.tile_pool(name="maskp", bufs=3)) if with_mask else None
        pp_mm1 = ep(tc.tile_pool(name="pmm1", bufs=2, space="PSUM"))
        pp_sc = ep(tc.tile_pool(name="psc", bufs=3, space="PSUM"))
        pp_pt = ep(tc.tile_pool(name="ppt", bufs=1, space="PSUM"))
        pp_o2 = ep(tc.tile_pool(name="po2", bufs=2, space="PSUM"))

        wT_sb = singles.tile([P, NCH, D], f16)
        nc.sync.dma_start(wT_sb[:], wT.rearrange("(dc di) e -> di dc e", di=P))
        if with_bias:
            wb_sb = singles.tile([P, NCH], f32)
            nc.sync.dma_start(wb_sb[:], wb)
        ident = singles.tile([P, P], f16)
        make_identity(nc, ident[:])

        # Software pipeline: for s-tile k, emit the score matmuls + softmax
        # (A) at step k, the P-transposes (T) at step k+1, and the output
        # matmul (M) at step k+2, so PE always has independent work while
        # the softmax chain (DVE max -> ACT exp -> DVE copy) of the
        # previous tile completes.
        recs = []

        def do_T(r):
            ptp = pp_pt.tile([P, L], f16)
            for j in range(NCH):
                nc.tensor.transpose(ptp[:, j * P:(j + 1) * P],
                                    r["p_sb"][:, j * P:(j + 1) * P], ident[:])
            pt_sb = ptbuf.tile([P, L], f16)
            nc.vector.tensor_copy(pt_sb[:], ptp[:])
            r["pt_sb"] = pt_sb

        def do_M(r):
            pt_sb = r["pt_sb"]
            h_sb = r["h_sb"]
            stat = r["stat"]
            out_sb = outp.tile([P, D], f32)
            for hh in range(2):
                ops = pp_o2.tile([P, 512], f32)
                for j in range(NCH):
                    nc.tensor.matmul(ops[:], pt_sb[:, j * P:(j + 1) * P],
                                     h_sb[:, j, hh * 512:(hh + 1) * 512],
                                     start=(j == 0), stop=(j == NCH - 1))
                # out = psum * (1/sum_exp) with per-row scale, fused cast
                nc.scalar.mul(out_sb[:, hh * 512:(hh + 1) * 512], ops[:],
                              mul=stat[:, 6:7])
            nc.sync.dma_start(out[r["b"], r["st"] * P:(r["st"] + 1) * P, :],
                              out_sb[:])

        for b in range(BPC):
            h_sb = batchp.tile([P, NCH, D], f16, tag="h")
            nc.sync.dma_start(h_sb[:],
                              h_[b].rearrange("(tc ti) e -> ti tc e", ti=P))
            hT_sb = batchp.tile([P, NCH, L], f16, tag="hT")
            nc.sync.dma_start(hT_sb[:],
                              hT[b].rearrange("(ec ei) t -> ei ec t", ei=P))
            for sc in range(L // SC):
                sIn_sb = sinp.tile([P, NCH, SC], f16)
                nc.sync.dma_start(
                    sIn_sb[:],
                    sT[b, :, sc * SC:(sc + 1) * SC].rearrange(
                        "(dc di) s -> di dc s", di=P))
                # mm1: proj^T[e, s] = sum_d W^T[d, e] * S^T[d, s]  (+ W_b)
                proj_sb = projp.tile([P, NCH, SC], f16)
                for ec in range(NCH):
                    ps = pp_mm1.tile([P, SC], f32)
                    for dc in range(NCH):
                        nc.tensor.matmul(ps[:], wT_sb[:, dc, ec * P:(ec + 1) * P],
                                         sIn_sb[:, dc, :],
                                         start=(dc == 0), stop=(dc == NCH - 1))
                    if with_bias:
                        nc.vector.tensor_scalar_add(proj_sb[:, ec, :], ps[:],
                                                    wb_sb[:, ec:ec + 1])
                    else:
                        nc.vector.tensor_copy(proj_sb[:, ec, :], ps[:])
                for st4 in range(SC // P):
                    st = sc * (SC // P) + st4
                    # ---- A: scores + softmax ----
                    stat = statp.tile([P, 8], f32)
                    p_sb = pbuf.tile([P, L], f16)
                    if with_mask:
                        m_sb = maskp.tile([P, L], f32)
                        nc.sync.dma_start(m_sb[:],
                                          msk[b, st * P:(st + 1) * P, :])
                    sps_tiles = []
                    for hh in range(2):
                        sps = pp_sc.tile([P, 512], f32)
                        for ec in range(NCH):
                            nc.tensor.matmul(
                                sps[:], proj_sb[:, ec, st4 * P:(st4 + 1) * P],
                                hT_sb[:, ec, hh * 512:(hh + 1) * 512],
                                start=(ec == 0), stop=(ec == NCH - 1))
                        if with_mask:
                            nc.vector.tensor_add(sps[:], sps[:],
                                                 m_sb[:, hh * 512:(hh + 1) * 512])
                        nc.vector.reduce_max(stat[:, hh:hh + 1], sps[:], axis=AX)
                        sps_tiles.append(sps)
                    nc.vector.reduce_max(stat[:, 2:3], stat[:, 0:2], axis=AX,
                                         negate=True)
                    for hh in range(2):
                        nc.scalar.activation(p_sb[:, hh * 512:(hh + 1) * 512],
                                             sps_tiles[hh][:], EXP,
                                             bias=stat[:, 2:3],
                                             accum_out=stat[:, 3 + hh:4 + hh])
                    nc.vector.reduce_sum(stat[:, 5:6], stat[:, 3:5], axis=AX)
                    nc.vector.reciprocal(stat[:, 6:7], stat[:, 5:6])

                    recs.append({"b": b, "st": st, "stat": stat, "p_sb": p_sb,
                                 "h_sb": h_sb})
                    if len(recs) >= 2:
                        do_T(recs[-2])
                    if len(recs) >= 3:
                        do_M(recs[-3])
        do_T(recs[-1])
        do_M(recs[-2])
        do_M(recs[-1])

    nc.compile()
    return nc


def _get_nc(with_mask: bool, with_bias: bool):
    key = (with_mask, with_bias)
    if key not in _nc_cache:
        _nc_cache[key] = _build_nc(with_mask, with_bias)
    return _nc_cache[key]


def kernel(S, H, pad_mask, W_w, W_b):
    from concourse import bass_utils

    S = np.asarray(S, dtype=np.float32)
    H = np.asarray(H, dtype=np.float32)
    pad_mask = np.asarray(pad_mask, dtype=np.float32)
    W_w = np.asarray(W_w, dtype=np.float32)
    W_b = np.asarray(W_b, dtype=np.float32)

    with_mask = bool(np.any(pad_mask))
    with_bias = bool(np.any(W_b))
    nc = _get_nc(with_mask, with_bias)

    S16 = S.astype(np.float16)
    H16 = H.astype(np.float16)
    ST = np.ascontiguousarray(S16.transpose(0, 2, 1))
    HT = np.ascontiguousarray(H16.transpose(0, 2, 1))
    H16 = np.ascontiguousarray(H16)
    wT = np.ascontiguousarray(W_w.astype(np.float16).T)
    wb = np.ascontiguousarray(W_b.reshape(NCH, P).T) if with_bias else None

    in_maps = []
    for c in range(NCORES):
        sl = slice(BPC * c, BPC * (c + 1))
        m = {"sT": ST[sl], "hT": HT[sl], "h": H16[sl], "wT": wT}
        if with_bias:
            m["wb"] = wb
        if with_mask:
            m["mask"] = np.ascontiguousarray(pad_mask[sl])
        in_maps.append(m)

    res = bass_utils.run_bass_kernel_spmd(nc, in_maps,
                                          core_ids=list(range(NCORES)))
    out = np.empty((B, L, D), dtype=np.float32)
    for c in range(NCORES):
        out[BPC * c:BPC * (c + 1)] = res.results[c]["out"]
    return out


# revision 6
# speedup vs baseline: 1.0736x; 1.0736x over previous
"""Bahdanau attention Trainium2 kernel.

Reference computation (per batch b):
    S_    = S[b] @ W_w.T + W_b          # [LS, D2]
    score = S_ @ H[b].T                 # [LS, LH]
    P     = softmax(score + pad_mask[b], axis=-1)
    out   = P @ H[b]                    # [LS, D2]

Sharding: data-parallel over batch B=16 across 8 NeuronCores (2 batches/core),
W replicated. All matmuls run in fp16 (full PE rate) with fp32 PSUM
accumulation; softmax statistics in fp32.

Host-side prep (inside kernel()): shard, cast to fp16, pre-transpose
S -> S^T (contraction over d needs d on partitions), W -> W^T, H -> H^T
(mm2 rhs), and keep H natural (mm3 rhs). The softmax probabilities are
transposed on-chip via PE transposes.
"""

import numpy as np

B, L, D = 16, 1024, 1024
NCORES = 8
BPC = B // NCORES  # batches per core
P = 128
NCH = D // P  # 128-row chunks per 1024 dim
SC = 512  # s-chunk width for the projection matmul
NTILE = BPC * (L // P)  # s-tiles per core across batches

_nc_cache = {}


def _build_nc(with_mask: bool, with_bias: bool):
    from contextlib import ExitStack

    import concourse.tile as tile
    from concourse import bacc, mybir
    from concourse.masks import make_identity

    f16 = mybir.dt.float16
    f32 = mybir.dt.float32
    AX = mybir.AxisListType.X
    EXP = mybir.ActivationFunctionType.Exp

    nc = bacc.Bacc("TRN2", target_bir_lowering=False, debug=False,
                   num_devices=NCORES)

    sT = nc.dram_tensor("sT", [BPC, D, L], f16, kind="ExternalInput").ap()
    hT = nc.dram_tensor("hT", [BPC, D, L], f16, kind="ExternalInput").ap()
    h_ = nc.dram_tensor("h", [BPC, L, D], f16, kind="ExternalInput").ap()
    wT = nc.dram_tensor("wT", [D, D], f16, kind="ExternalInput").ap()
    wb = (nc.dram_tensor("wb", [P, NCH], f32, kind="ExternalInput").ap()
          if with_bias else None)
    msk = (nc.dram_tensor("mask", [BPC, L, L], f32, kind="ExternalInput").ap()
           if with_mask else None)
    out = nc.dram_tensor("out", [BPC, L, D], f32, kind="ExternalOutput").ap()

    with tile.TileContext(nc) as tc, ExitStack() as ctx:
        ep = ctx.enter_context
        singles = ep(tc.tile_pool(name="singles", bufs=1))
        batchp = ep(tc.tile_pool(name="batchp", bufs=2))
        sinp = ep(tc.tile_pool(name="sin", bufs=2))
        projp = ep(tc.tile_pool(name="proj", bufs=2))
        pbuf = ep(tc.tile_pool(name="pbuf", bufs=2))
        ptbuf = ep(tc.tile_pool(name="ptbuf", bufs=2))
        outp = ep(tc.tile_pool(name="outp", bufs=3))
        statp = ep(tc.tile_pool(name="statp", bufs=4))
        maskp = ep(tc.tile_pool(name="maskp", bufs=3)) if with_mask else None
        pp_mm1 = ep(tc.tile_pool(name="pmm1", bufs=2, space="PSUM"))
        pp_sc = ep(tc.tile_pool(name="psc", bufs=3, space="PSUM"))
        pp_pt = ep(tc.tile_pool(name="ppt", bufs=1, space="PSUM"))
        pp_o2 = ep(tc.tile_pool(name="po2", bufs=2, space="PSUM"))

        ident = singles.tile([P, P], f16)
        make_identity(nc, ident[:])

        # Interleave the W^T and first S^T chunk loads so the projection
        # matmul can start as soon as its first d-chunks land, instead of
        # waiting for every input tensor queued ahead of it.
        wT_sb = singles.tile([P, NCH, D], f16)
        wT_src = wT.rearrange("(dc di) e -> di dc e", di=P)
        sin0_b0 = sinp.tile([P, NCH, SC], f16)
        sin0_src = sT[0, :, 0:SC].rearrange("(dc di) s -> di dc s", di=P)
        for i in range(4):
            dsl = slice(2 * i, 2 * i + 2)
            nc.sync.dma_start(wT_sb[:, dsl, :], wT_src[:, dsl, :])
            nc.sync.dma_start(sin0_b0[:, dsl, :], sin0_src[:, dsl, :])
        if with_bias:
            wb_sb = singles.tile([P, NCH], f32)
            nc.sync.dma_start(wb_sb[:], wb)

        # HAM warmup: keep the PE busy with throwaway matmuls while the
        # first input chunks stream in, so the real matmuls start at the
        # un-throttled 2.4 GHz clock (the activity monitor needs ~3.4us of
        # sustained work before it lifts the 1.2 GHz cold throttle).
        warm_ps = pp_mm1.tile([P, P], f32, tag="ps")
        for _ in range(40):
            nc.tensor.matmul(warm_ps[:], ident[:], ident[:],
                             start=True, stop=True)

        # Software pipeline: for s-tile k, emit the score matmuls + softmax
        # (A) at step k, the P-transposes (T) at step k+1, and the output
        # matmul (M) at step k+2, so PE always has independent work while
        # the softmax chain (DVE max -> ACT exp -> DVE copy) of the
        # previous tile completes.
        recs = []

        def do_T(r):
            ptp = pp_pt.tile([P, L], f16)
            for j in range(NCH):
                nc.tensor.transpose(ptp[:, j * P:(j + 1) * P],
                                    r["p_sb"][:, j * P:(j + 1) * P], ident[:])
            pt_sb = ptbuf.tile([P, L], f16)
            nc.vector.tensor_copy(pt_sb[:], ptp[:])
            r["pt_sb"] = pt_sb

        def do_M(r):
            pt_sb = r["pt_sb"]
            h_sb = r["h_sb"]
            stat = r["stat"]
            out_sb = outp.tile([P, D], f32)
            for hh in range(2):
                ops = pp_o2.tile([P, 512], f32)
                for j in range(NCH):
                    nc.tensor.matmul(ops[:], pt_sb[:, j * P:(j + 1) * P],
                                     h_sb[:, j, hh * 512:(hh + 1) * 512],
                                     start=(j == 0), stop=(j == NCH - 1))
                # out = psum * (1/sum_exp) with per-row scale, fused cast
                nc.scalar.mul(out_sb[:, hh * 512:(hh + 1) * 512], ops[:],
                              mul=stat[:, 6:7])
            nc.sync.dma_start(out[r["b"], r["st"] * P:(r["st"] + 1) * P, :],
                              out_sb[:])

        for b in range(BPC):
            def load_sin(sc, b=b):
                t = sinp.tile([P, NCH, SC], f16)
                nc.sync.dma_start(
                    t[:],
                    sT[b, :, sc * SC:(sc + 1) * SC].rearrange(
                        "(dc di) s -> di dc s", di=P))
                return t

            sins = [sin0_b0 if b == 0 else load_sin(0)]
            hT_sb = batchp.tile([P, NCH, L], f16, tag="hT")
            nc.sync.dma_start(hT_sb[:],
                              hT[b].rearrange("(ec ei) t -> ei ec t", ei=P))
            h_sb = batchp.tile([P, NCH, D], f16, tag="h")
            nc.sync.dma_start(h_sb[:],
                              h_[b].rearrange("(tc ti) e -> ti tc e", ti=P))
            for sc in range(1, L // SC):
                sins.append(load_sin(sc))
            for sc in range(L // SC):
                sIn_sb = sins[sc]
                # mm1: proj^T[e, s] = sum_d W^T[d, e] * S^T[d, s]  (+ W_b)
                proj_sb = projp.tile([P, NCH, SC], f16)
                for ec in range(NCH):
                    ps = pp_mm1.tile([P, SC], f32)
                    for dc in range(NCH):
                        nc.tensor.matmul(ps[:], wT_sb[:, dc, ec * P:(ec + 1) * P],
                                         sIn_sb[:, dc, :],
                                         start=(dc == 0), stop=(dc == NCH - 1))
                    if with_bias:
                        nc.vector.tensor_scalar_add(proj_sb[:, ec, :], ps[:],
                                                    wb_sb[:, ec:ec + 1])
                    else:
                        nc.vector.tensor_copy(proj_sb[:, ec, :], ps[:])
                for st4 in range(SC // P):
                    st = sc * (SC // P) + st4
                    # ---- A: scores + softmax ----
                    stat = statp.tile([P, 8], f32)
                    p_sb = pbuf.tile([P, L], f16)
                    if with_mask:
                        m_sb = maskp.tile([P, L], f32)
                        nc.sync.dma_start(m_sb[:],
                                          msk[b, st * P:(st + 1) * P, :])
                    sps_tiles = []
                    for hh in range(2):
                        sps = pp_sc.tile([P, 512], f32)
                        for ec in range(NCH):
                            nc.tensor.matmul(
                                sps[:], proj_sb[:, ec, st4 * P:(st4 + 1) * P],
                                hT_sb[:, ec, hh * 512:(hh + 1) * 512],
                                start=(ec == 0), stop=(ec == NCH - 1))
                        if with_mask:
                            nc.vector.tensor_add(sps[:], sps[:],
                                                 m_sb[:, hh * 512:(hh + 1) * 512])
                        nc.vector.reduce_max(stat[:, hh:hh + 1], sps[:], axis=AX)
                        sps_tiles.append(sps)
                    nc.vector.reduce_max(stat[:, 2:3], stat[:, 0:2], axis=AX,
                                         negate=True)
                    for hh in range(2):
                        nc.scalar.activation(p_sb[:, hh * 512:(hh + 1) * 512],
                                             sps_tiles[hh][:], EXP,
                                             bias=stat[:, 2:3],
                                             accum_out=stat[:, 3 + hh:4 + hh])
                    nc.vector.reduce_sum(stat[:, 5:6], stat[:, 3:5], axis=AX)
                    nc.vector.reciprocal(stat[:, 6:7], stat[:, 5:6])

                    recs.append({"b": b, "st": st, "stat": stat, "p_sb": p_sb,
                                 "h_sb": h_sb})
                    if len(recs) >= 2:
                        do_T(recs[-2])
                    if len(recs) >= 3:
                        do_M(recs[-3])
        do_M(recs[-2])
        do_T(recs[-1])
        do_M(recs[-1])

    nc.compile()
    return nc


def _get_nc(with_mask: bool, with_bias: bool):
    key = (with_mask, with_bias)
    if key not in _nc_cache:
        _nc_cache[key] = _build_nc(with_mask, with_bias)
    return _nc_cache[key]


def kernel(S, H, pad_mask, W_w, W_b):
    from concourse import bass_utils

    S = np.asarray(S, dtype=np.float32)
    H = np.asarray(H, dtype=np.float32)
    pad_mask = np.asarray(pad_mask, dtype=np.float32)
    W_w = np.asarray(W_w, dtype=np.float32)
    W_b = np.asarray(W_b, dtype=np.float32)

    with_mask = bool(np.any(pad_mask))
    with_bias = bool(np.any(W_b))
    nc = _get_nc(with_mask, with_bias)

    S16 = S.astype(np.float16)
    H16 = H.astype(np.float16)
    ST = np.ascontiguousarray(S16.transpose(0, 2, 1))
    HT = np.ascontiguousarray(H16.transpose(0, 2, 1))
    H16 = np.ascontiguousarray(H16)
    wT = np.ascontiguousarray(W_w.astype(np.float16).T)
    wb = np.ascontiguousarray(W_b.reshape(NCH, P).T) if with_bias else None

    in_maps = []
    for c in range(NCORES):
        sl = slice(BPC * c, BPC * (c + 1))
        m = {"sT": ST[sl], "hT": HT[sl], "h": H16[sl], "wT": wT}
        if with_bias:
            m["wb"] = wb
        if with_mask:
            m["mask"] = np.ascontiguousarray(pad_mask[sl])
        in_maps.append(m)

    res = bass_utils.run_bass_kernel_spmd(nc, in_maps,
                                          core_ids=list(range(NCORES)))
    out = np.empty((B, L, D), dtype=np.float32)
    for c in range(NCORES):
        out[BPC * c:BPC * (c + 1)] = res.results[c]["out"]
    return out


# revision 11
# speedup vs baseline: 1.0743x; 1.0006x over previous
"""Bahdanau attention Trainium2 kernel.

Reference computation (per batch b):
    S_    = S[b] @ W_w.T + W_b          # [LS, D2]
    score = S_ @ H[b].T                 # [LS, LH]
    P     = softmax(score + pad_mask[b], axis=-1)
    out   = P @ H[b]                    # [LS, D2]

Sharding: data-parallel over batch B=16 across 8 NeuronCores (2 batches/core),
W replicated. All matmuls run in fp16 (full PE rate) with fp32 PSUM
accumulation; softmax statistics in fp32.

Host-side prep (inside kernel()): shard, cast to fp16, pre-transpose
S -> S^T (contraction over d needs d on partitions), W -> W^T, H -> H^T
(mm2 rhs), and keep H natural (mm3 rhs). The softmax probabilities are
transposed on-chip via PE transposes.
"""

import numpy as np

B, L, D = 16, 1024, 1024
NCORES = 8
BPC = B // NCORES  # batches per core
P = 128
NCH = D // P  # 128-row chunks per 1024 dim
SC = 512  # s-chunk width for the projection matmul
NTILE = BPC * (L // P)  # s-tiles per core across batches

_nc_cache = {}


def _build_nc(with_mask: bool, with_bias: bool):
    from contextlib import ExitStack

    import concourse.tile as tile
    from concourse import bacc, mybir
    from concourse.masks import make_identity

    f16 = mybir.dt.float16
    f32 = mybir.dt.float32
    AX = mybir.AxisListType.X
    EXP = mybir.ActivationFunctionType.Exp

    nc = bacc.Bacc("TRN2", target_bir_lowering=False, debug=False,
                   num_devices=NCORES)

    sT = nc.dram_tensor("sT", [BPC, D, L], f16, kind="ExternalInput").ap()
    hT = nc.dram_tensor("hT", [BPC, D, L], f16, kind="ExternalInput").ap()
    h_ = nc.dram_tensor("h", [BPC, L, D], f16, kind="ExternalInput").ap()
    # W^T pre-arranged on host as [ec, di, dc, ei] so each 256KB e-slice is
    # one contiguous DMA and the projection matmul can start after the first
    # slice instead of the whole 2MB.
    wT = nc.dram_tensor("wT", [NCH, P, NCH, P], f16, kind="ExternalInput").ap()
    wb = (nc.dram_tensor("wb", [P, NCH], f32, kind="ExternalInput").ap()
          if with_bias else None)
    msk = (nc.dram_tensor("mask", [BPC, L, L], f32, kind="ExternalInput").ap()
           if with_mask else None)
    out = nc.dram_tensor("out", [BPC, L, D], f32, kind="ExternalOutput").ap()

    with tile.TileContext(nc) as tc, ExitStack() as ctx:
        ep = ctx.enter_context
        singles = ep(tc.tile_pool(name="singles", bufs=1))
        batchp = ep(tc.tile_pool(name="batchp", bufs=2))
        sinp = ep(tc.tile_pool(name="sin", bufs=2))
        projp = ep(tc.tile_pool(name="proj", bufs=2))
        pbuf = ep(tc.tile_pool(name="pbuf", bufs=2))
        ptbuf = ep(tc.tile_pool(name="ptbuf", bufs=2))
        outp = ep(tc.tile_pool(name="outp", bufs=3))
        statp = ep(tc.tile_pool(name="statp", bufs=4))
        maskp = ep(tc.tile_pool(name="maskp", bufs=3)) if with_mask else None
        pp_mm1 = ep(tc.tile_pool(name="pmm1", bufs=2, space="PSUM"))
        pp_sc = ep(tc.tile_pool(name="psc", bufs=3, space="PSUM"))
        pp_pt = ep(tc.tile_pool(name="ppt", bufs=1, space="PSUM"))
        pp_o2 = ep(tc.tile_pool(name="po2", bufs=2, space="PSUM"))

        ident = singles.tile([P, P], f16)
        make_identity(nc, ident[:])

        # Load order is the HBM critical path: the first projection matmul
        # group needs all of S^T chunk 0 (1MB) plus one W^T e-slice (256KB);
        # later e-slices arrive faster than the PE consumes them.
        sin0_b0 = sinp.tile([P, NCH, SC], f16)
        sin0_src = sT[0, :, 0:SC].rearrange("(dc di) s -> di dc s", di=P)
        for i in range(4):
            dsl = slice(2 * i, 2 * i + 2)
            nc.sync.dma_start(sin0_b0[:, dsl, :], sin0_src[:, dsl, :])
        wT_sb = singles.tile([P, NCH, NCH, P], f16)  # [di, ec, dc, ei]
        for ec in range(NCH):
            nc.sync.dma_start(wT_sb[:, ec], wT[ec])
        if with_bias:
            wb_sb = singles.tile([P, NCH], f32)
            nc.sync.dma_start(wb_sb[:], wb)

        # HAM warmup: keep the PE busy with throwaway matmuls while the
        # first input chunks stream in, so the real matmuls start at the
        # un-throttled 2.4 GHz clock (the activity monitor needs ~3.4us of
        # sustained work before it lifts the 1.2 GHz cold throttle).
        warm_ps = pp_mm1.tile([P, P], f32, tag="ps")
        for _ in range(36):
            nc.tensor.matmul(warm_ps[:], ident[:], ident[:],
                             start=True, stop=True)

        # Software pipeline: for s-tile k, emit the score matmuls + softmax
        # (A) at step k, the P-transposes (T) at step k+1, and the output
        # matmul (M) at step k+2, so PE always has independent work while
        # the softmax chain (DVE max -> ACT exp -> DVE copy) of the
        # previous tile completes.
        recs = []

        def do_T(r):
            ptp = pp_pt.tile([P, L], f16)
            for j in range(NCH):
                nc.tensor.transpose(ptp[:, j * P:(j + 1) * P],
                                    r["p_sb"][:, j * P:(j + 1) * P], ident[:])
            pt_sb = ptbuf.tile([P, L], f16)
            nc.vector.tensor_copy(pt_sb[:], ptp[:])
            r["pt_sb"] = pt_sb

        def do_M(r):
            pt_sb = r["pt_sb"]
            h_sb = r["h_sb"]
            stat = r["stat"]
            out_sb = outp.tile([P, D], f32)
            for hh in range(2):
                ops = pp_o2.tile([P, 512], f32)
                for j in range(NCH):
                    nc.tensor.matmul(ops[:], pt_sb[:, j * P:(j + 1) * P],
                                     h_sb[:, j, hh * 512:(hh + 1) * 512],
                                     start=(j == 0), stop=(j == NCH - 1))
                # out = psum * (1/sum_exp) with per-row scale, fused cast
                nc.scalar.mul(out_sb[:, hh * 512:(hh + 1) * 512], ops[:],
                              mul=stat[:, 6:7])
                nc.sync.dma_start(
                    out[r["b"], r["st"] * P:(r["st"] + 1) * P,
                        hh * 512:(hh + 1) * 512],
                    out_sb[:, hh * 512:(hh + 1) * 512])

        for b in range(BPC):
            def load_sin(sc, b=b):
                t = sinp.tile([P, NCH, SC], f16)
                nc.sync.dma_start(
                    t[:],
                    sT[b, :, sc * SC:(sc + 1) * SC].rearrange(
                        "(dc di) s -> di dc s", di=P))
                return t

            sins = [sin0_b0 if b == 0 else load_sin(0)]
            hT_sb = batchp.tile([P, NCH, L], f16, tag="hT")
            nc.sync.dma_start(hT_sb[:],
                              hT[b].rearrange("(ec ei) t -> ei ec t", ei=P))
            h_sb = batchp.tile([P, NCH, D], f16, tag="h")
            nc.sync.dma_start(h_sb[:],
                              h_[b].rearrange("(tc ti) e -> ti tc e", ti=P))
            for sc in range(1, L // SC):
                sins.append(load_sin(sc))
            for sc in range(L // SC):
                sIn_sb = sins[sc]
                # mm1: proj^T[e, s] = sum_d W^T[d, e] * S^T[d, s]  (+ W_b)
                proj_sb = projp.tile([P, NCH, SC], f16)
                for ec in range(NCH):
                    ps = pp_mm1.tile([P, SC], f32)
                    for dc in range(NCH):
                        nc.tensor.matmul(ps[:], wT_sb[:, ec, dc, :],
                                         sIn_sb[:, dc, :],
                                         start=(dc == 0), stop=(dc == NCH - 1))
                    if with_bias:
                        nc.vector.tensor_scalar_add(proj_sb[:, ec, :], ps[:],
                                                    wb_sb[:, ec:ec + 1])
                    else:
                        nc.vector.tensor_copy(proj_sb[:, ec, :], ps[:])
                for st4 in range(SC // P):
                    st = sc * (SC // P) + st4
                    # ---- A: scores + softmax ----
                    stat = statp.tile([P, 8], f32)
                    p_sb = pbuf.tile([P, L], f16)
                    if with_mask:
                        m_sb = maskp.tile([P, L], f32)
                        nc.sync.dma_start(m_sb[:],
                                          msk[b, st * P:(st + 1) * P, :])
                    sps_tiles = []
                    for hh in range(2):
                        sps = pp_sc.tile([P, 512], f32)
                        for ec in range(NCH):
                            nc.tensor.matmul(
                                sps[:], proj_sb[:, ec, st4 * P:(st4 + 1) * P],
                                hT_sb[:, ec, hh * 512:(hh + 1) * 512],
                                start=(ec == 0), stop=(ec == NCH - 1))
                        if with_mask:
                            nc.vector.tensor_add(sps[:], sps[:],
                                                 m_sb[:, hh * 512:(hh + 1) * 512])
                        nc.vector.reduce_max(stat[:, hh:hh + 1], sps[:], axis=AX)
                        sps_tiles.append(sps)
                    nc.vector.reduce_max(stat[:, 2:3], stat[:, 0:2], axis=AX,
                                         negate=True)
                    for hh in range(2):
                        nc.scalar.activation(p_sb[:, hh * 512:(hh + 1) * 512],
                                             sps_tiles[hh][:], EXP,
                                             bias=stat[:, 2:3],
                                             accum_out=stat[:, 3 + hh:4 + hh])
                    nc.vector.reduce_sum(stat[:, 5:6], stat[:, 3:5], axis=AX)
                    nc.vector.reciprocal(stat[:, 6:7], stat[:, 5:6])

                    recs.append({"b": b, "st": st, "stat": stat, "p_sb": p_sb,
                                 "h_sb": h_sb})
                    if len(recs) >= 2:
                        do_T(recs[-2])
                    if len(recs) >= 3:
                        do_M(recs[-3])
        do_M(recs[-2])
        do_T(recs[-1])
        do_M(recs[-1])

    nc.compile()
    return nc


def _get_nc(with_mask: bool, with_bias: bool):
    key = (with_mask, with_bias)
    if key not in _nc_cache:
        _nc_cache[key] = _build_nc(with_mask, with_bias)
    return _nc_cache[key]


def kernel(S, H, pad_mask, W_w, W_b):
    from concourse import bass_utils

    S = np.asarray(S, dtype=np.float32)
    H = np.asarray(H, dtype=np.float32)
    pad_mask = np.asarray(pad_mask, dtype=np.float32)
    W_w = np.asarray(W_w, dtype=np.float32)
    W_b = np.asarray(W_b, dtype=np.float32)

    with_mask = bool(np.any(pad_mask))
    with_bias = bool(np.any(W_b))
    nc = _get_nc(with_mask, with_bias)

    S16 = S.astype(np.float16)
    H16 = H.astype(np.float16)
    ST = np.ascontiguousarray(S16.transpose(0, 2, 1))
    HT = np.ascontiguousarray(H16.transpose(0, 2, 1))
    H16 = np.ascontiguousarray(H16)
    # [d, e] -> [ec, di, dc, ei] (e-slice-major, contiguous per slice)
    wT = np.ascontiguousarray(
        W_w.astype(np.float16).T.reshape(NCH, P, NCH, P).transpose(2, 1, 0, 3))
    wb = np.ascontiguousarray(W_b.reshape(NCH, P).T) if with_bias else None

    in_maps = []
    for c in range(NCORES):
        sl = slice(BPC * c, BPC * (c + 1))
        m = {"sT": ST[sl], "hT": HT[sl], "h": H16[sl], "wT": wT}
        if with_bias:
            m["wb"] = wb
        if with_mask:
            m["mask"] = np.ascontiguousarray(pad_mask[sl])
        in_maps.append(m)

    res = bass_utils.run_bass_kernel_spmd(nc, in_maps,
                                          core_ids=list(range(NCORES)))
    out = np.empty((B, L, D), dtype=np.float32)
    for c in range(NCORES):
        out[BPC * c:BPC * (c + 1)] = res.results[c]["out"]
    return out
